# revision 4
# baseline (speedup 1.0000x reference)
"""Trainium2 Bass kernel for CalibratedProjectiveLinear (QINS log-quantized linear).

y = (x @ W^T + bias) * scale, with W reconstructed elementwise from a
log-scale uint8 encoding: W[o,i] = sign[o,i] * exp(log_min + (255-stored[o,i])/254
* (log_max-log_min)).

Sharding: column-parallel over out_features across 8 cores. x is replicated
(passed transposed so the contraction dim lands on SBUF partitions).

The int32 stored/sign tensors carry 1 byte of information per element
(stored in [1,255], sign in {-1,+1}), so the host shards AND narrows them to
uint8 / int8 during its layout transform — per-core weight traffic drops
45.1 MB -> 11.3 MB, moving the kernel from DMA-bound (146 us) to PE-bound
(~75 us: 176k matmul rows at 1 row/cycle, 2.4 GHz).

Device pipeline per core, per contraction super-chunk (CHUNK x 128 rows):
  linear DMA uint8 stored / int8 sign -> ACT: exp(c1*stored + c0) uint8->bf16
  -> DVE: multiply by sign (int8 operand) -> PE: bf16 matmuls accumulating
  into PSUM over the 32 contraction chunks (bf16 streams 1 row/cycle, same
  as fp32r at free-dim 512, and halves SBUF + DVE cost; product error
  ~2.3e-3 vs the 2e-2 gate). Bias and per-channel scale are both applied
  during the PSUM->SBUF evacuation (DVE tensor_scalar: acc*scale +
  bias*scale with per-partition vectors), eliminating the baseline's
  rank-1 bias matmuls. All output stores are held in SBUF and issued at
  the end of the body so the weight-read stream is never interleaved with
  HBM writes.
"""

import numpy as np

import concourse.bass as bass
import concourse.mybir as mybir
from concourse import tile
from concourse.bass_utils import run_bass_kernel_spmd

B, IN, OUT = 512, 4096, 11008
N_CORES = 8
O_SH = OUT // N_CORES            # 1376 out-features per core
K_TILES = IN // 128              # 32 contraction chunks
O_TILE_WIDTHS = [128] * (O_SH // 128) + ([O_SH % 128] if O_SH % 128 else [])
N_OT = len(O_TILE_WIDTHS)        # 11 (10x128 + 96)
O_GROUPS = [list(range(0, 4)), list(range(4, 8)), list(range(8, N_OT))]
CHUNK = 4                        # contraction chunks per weight DMA
FP32 = mybir.dt.float32
BF16 = mybir.dt.bfloat16
UINT8 = mybir.dt.uint8
INT8 = mybir.dt.int8

_COMPILED = {}


def _group_geometry():
    o_offs = np.cumsum([0] + O_TILE_WIDTHS).tolist()
    geo = []
    blk_off = 0
    for group in O_GROUPS:
        g0 = o_offs[group[0]]
        gw = o_offs[group[-1] + 1] - g0
        geo.append((group, g0, gw, blk_off))
        blk_off += IN * gw
    return o_offs, geo


def _split_multi_waits(nc: bass.Bass) -> int:
    """The walrus build in this container accepts at most ONE sync wait per
    instruction; Tile freely emits several. Split extras into single-wait
    NoOps on the same engine, inserted just before the instruction
    (semantically identical: all waits must pass before it executes)."""
    n_split = 0
    for blk in nc.main_func.blocks:
        new_insts = []
        for inst in blk.instructions:
            si = inst.sync_info
            if si is not None and len(si.on_wait) > 1:
                waits = list(si.on_wait)
                for w in waits[:-1]:
                    nop = mybir.InstNoOp(
                        name=nc.get_next_instruction_name(), ins=[], outs=[])
                    nop.engine = inst.engine
                    nop.sync_info = mybir.SyncInfo(on_wait=[w], on_update=[])
                    nc.register_instruction(nop)
                    new_insts.append(nop)
                    n_split += 1
                inst.sync_info = mybir.SyncInfo(
                    on_wait=[waits[-1]], on_update=list(si.on_update))
            new_insts.append(inst)
        blk.instructions = new_insts
    return n_split


def _build(c0: float, c1: float, repeat: int = 1, variant: str = "wend",
           stage_bufs: int = 3, unroll: bool = False) -> bass.Bass:
    nc = bass.Bass()
    storedB = nc.dram_tensor("storedB", [IN * O_SH], UINT8, kind="ExternalInput")
    signB = nc.dram_tensor("signB", [IN * O_SH], INT8, kind="ExternalInput")
    xT = nc.dram_tensor("xT", [IN, B], FP32, kind="ExternalInput")
    scale_m = nc.dram_tensor("scale_m", [128, N_OT], FP32, kind="ExternalInput")
    bsc_m = nc.dram_tensor("bsc_m", [128, N_OT], FP32, kind="ExternalInput")
    out = nc.dram_tensor("out", [O_SH, B], FP32, kind="ExternalOutput")

    with tile.TileContext(nc) as tc:
        with (
            tc.tile_pool(name="consts", bufs=1) as consts,
            tc.tile_pool(name="stage", bufs=stage_bufs) as stage,
            tc.tile_pool(name="resp", bufs=3) as resp,
            tc.tile_pool(name="xstage", bufs=2) as xstage,
            tc.tile_pool(name="psum", bufs=1, space="PSUM") as psum,
        ):
            c0_t = consts.tile([128, 1], FP32)
            nc.vector.memset(c0_t[:], c0)
            scale_t = consts.tile([128, N_OT], FP32)
            nc.sync.dma_start(scale_t[:], scale_m[:])
            bsc_t = consts.tile([128, N_OT], FP32)
            nc.sync.dma_start(bsc_t[:], bsc_m[:])

            x_tiles = {}

            def get_x(i):
                # Lazy: emitted at first use so the weight-stream DMAs are
                # not queued behind the full 8.4 MB x preload at kernel
                # start. For repeat>1 all tiles are pre-emitted outside the
                # loop (below), so the loop body slope measures steady state.
                if i not in x_tiles:
                    xf = xstage.tile([128, B], FP32, tag="xf", name=f"xf_{i}")
                    nc.sync.dma_start(xf[:], xT[i * 128:(i + 1) * 128, :])
                    xt = consts.tile([128, B], BF16, tag=f"x{i}", name=f"x_{i}")
                    nc.vector.tensor_copy(xt[:], xf[:])
                    x_tiles[i] = xt
                return x_tiles[i]

            if repeat != 1:
                for i in range(K_TILES):
                    get_x(i)

            o_offs, geo = _group_geometry()

            def body():
                emit_groups(nc, o_offs, geo, storedB, signB, out,
                            get_x, scale_t, bsc_t, c0_t, c1,
                            stage, resp, psum, variant)

            if repeat == 1:
                body()
            elif unroll:
                for _ in range(repeat):
                    body()
            else:
                with tc.For_i(0, repeat, 1):
                    body()

    _split_multi_waits(nc)
    nc.finalize()
    return nc


def emit_groups(nc, o_offs, geo, storedB, signB, out, get_x,
                scale_t, bsc_t, c0_t, c1, stage, resp, psum, variant="wend"):
    late_stores = []

    def make_tail(group, accs):
        # group output path: per-channel scale AND bias applied during the
        # PSUM -> SBUF evacuation (DVE tensor_scalar: acc*scale + bias*scale
        # with per-partition vectors), then store. Emitted AFTER the next
        # group's pipeline has started so in-order engine queues never stall.
        # variant "wend": stores are held until the end of the body so the
        # weight-read stream is never interleaved with HBM writes.
        def tail():
            for t in group:
                tw = O_TILE_WIDTHS[t]
                oo = o_offs[t]
                if variant == "nout":
                    continue
                tag = f"res{t}" if variant == "wend" else "res"
                res = resp.tile([128, B], FP32, tag=tag, name=f"res_{t}")
                nc.vector.tensor_scalar(res[:tw, :], accs[t][:],
                                        scale_t[:tw, t:t + 1],
                                        bsc_t[:tw, t:t + 1],
                                        mybir.AluOpType.mult,
                                        mybir.AluOpType.add)
                if variant == "wend":
                    late_stores.append((oo, tw, res))
                else:
                    nc.sync.dma_start(out[oo:oo + tw, :], res[:tw, :])
        return tail

    pending_tail = None
    for group, g0, gw, blk in geo:
        if variant != "nope":
            accs = {t: psum.tile([O_TILE_WIDTHS[t], B], FP32,
                                 name=f"acc_{t}", tag=f"acc{t % 8}")
                    for t in group}
        for ib in range(K_TILES // CHUNK):
            # one fully-linear DMA covering CHUNK contraction chunks
            span = CHUNK * 128 * gw
            src_st = storedB[blk + ib * span: blk + (ib + 1) * span]
            src_sg = signB[blk + ib * span: blk + (ib + 1) * span]
            st = stage.tile([128, CHUNK, gw], UINT8, tag="st")
            nc.sync.dma_start(st[:], src_st.rearrange("(p a b) -> p a b",
                                                      p=128, b=gw))
            sg = stage.tile([128, CHUNK, gw], INT8, tag="sg")
            nc.sync.dma_start(sg[:], src_sg.rearrange("(p a b) -> p a b",
                                                      p=128, b=gw))
            if variant == "dma":
                continue
            wmag = stage.tile([128, CHUNK, gw], BF16, tag="wmag")
            nc.scalar.activation(wmag[:], st[:], mybir.ActivationFunctionType.Exp,
                                 bias=c0_t[:], scale=c1)
            w = stage.tile([128, CHUNK, gw], BF16, tag="w")
            nc.vector.tensor_mul(w[:], wmag[:], sg[:])
            if variant == "nope":
                continue
            for j in range(CHUNK):
                i = ib * CHUNK + j
                for t in group:
                    tw = O_TILE_WIDTHS[t]
                    toff = o_offs[t] - g0
                    nc.tensor.matmul(
                        accs[t][:],
                        w[:, j, toff:toff + tw],
                        get_x(i)[:],
                        start=(i == 0), stop=(i == K_TILES - 1),
                    )
            if ib == 1 and pending_tail is not None:
                pending_tail()
                pending_tail = None
        if variant in ("nope", "dma"):
            continue
        pending_tail = make_tail(group, accs)
    if pending_tail is not None:
        pending_tail()
    for oo, tw, res in late_stores:
        nc.sync.dma_start(out[oo:oo + tw, :], res[:tw, :])


def _blocked(mT: np.ndarray) -> np.ndarray:
    """[IN, O_SH] -> flat layout where every DMA tile [128, CHUNK*gw] is
    per-partition contiguous: for each group, for each super-chunk ib,
    a [128][CHUNK*gw] block with partition p holding rows
    (ib*CHUNK+j)*128 + p for j in 0..CHUNK."""
    _, geo = _group_geometry()
    parts = []
    for _, g0, gw, _ in geo:
        gcols = mT[:, g0:g0 + gw]              # [IN, gw]
        blk = gcols.reshape(K_TILES // CHUNK, CHUNK, 128, gw)
        parts.append(np.ascontiguousarray(blk.transpose(0, 2, 1, 3)).ravel())
    return np.concatenate(parts)


def kernel(x, stored, sign, log_min, log_max, scale, bias):
    log_min = float(np.asarray(log_min))
    log_max = float(np.asarray(log_max))
    # exp(log_min + (255 - s)/254 * d) == exp(c0 + c1*s)
    d = log_max - log_min
    c1 = -d / 254.0
    c0 = log_min + 255.0 * d / 254.0

    key = (c0, c1)
    if key not in _COMPILED:
        _COMPILED[key] = _build(c0, c1)
    nc = _COMPILED[key]

    xT = np.ascontiguousarray(np.asarray(x, dtype=np.float32).T)
    stored_u8 = np.asarray(stored, dtype=np.uint8)       # values in [1,255]
    sign_i8 = np.asarray(sign, dtype=np.int8)            # values in {-1,+1}
    scale = np.asarray(scale, dtype=np.float32)
    bias = np.asarray(bias, dtype=np.float32)

    in_maps = []
    for c in range(N_CORES):
        o0, o1 = c * O_SH, (c + 1) * O_SH
        scale_pad = np.ones(N_OT * 128, dtype=np.float32)
        scale_pad[:O_SH] = scale[o0:o1]
        bsc_pad = np.zeros(N_OT * 128, dtype=np.float32)
        bsc_pad[:O_SH] = bias[o0:o1] * scale[o0:o1]
        in_maps.append({
            "storedB": _blocked(stored_u8[o0:o1].T),
            "signB": _blocked(sign_i8[o0:o1].T),
            "xT": xT,
            "scale_m": np.ascontiguousarray(scale_pad.reshape(N_OT, 128).T),
            "bsc_m": np.ascontiguousarray(bsc_pad.reshape(N_OT, 128).T),
        })

    global _last_in_maps
    _last_in_maps = in_maps
    res = run_bass_kernel_spmd(nc, in_maps, list(range(N_CORES)))
    yT = np.concatenate([res.results[c]["out"] for c in range(N_CORES)], axis=0)
    return np.ascontiguousarray(yT.T)


# revision 5
# speedup vs baseline: 1.1184x; 1.1184x over previous
"""Trainium2 Bass kernel for CalibratedProjectiveLinear (QINS log-quantized linear).

y = (x @ W^T + bias) * scale, with W reconstructed elementwise from a
log-scale uint8 encoding: W[o,i] = sign[o,i] * exp(log_min + (255-stored[o,i])/254
* (log_max-log_min)).

Sharding: column-parallel over out_features across 8 cores. x is replicated
(passed transposed so the contraction dim lands on SBUF partitions).

The int32 stored/sign tensors carry 1 byte of information per element
(stored in [1,255], sign in {-1,+1}), so the host shards AND narrows them to
uint8 / int8 during its layout transform — per-core weight traffic drops
45.1 MB -> 11.3 MB, moving the kernel from DMA-bound (146 us) to PE-bound
(~75 us: 176k matmul rows at 1 row/cycle, 2.4 GHz).

Device pipeline per core, per contraction super-chunk (CHUNK x 128 rows):
  linear DMA uint8 stored / int8 sign -> ACT: exp(c1*stored + c0) uint8->bf16
  -> DVE: multiply by sign (int8 operand) -> PE: bf16 matmuls accumulating
  into PSUM over the 32 contraction chunks (bf16 streams 1 row/cycle, same
  as fp32r at free-dim 512, and halves SBUF + DVE cost; product error
  ~2.3e-3 vs the 2e-2 gate). Bias and per-channel scale are both applied
  during the PSUM->SBUF evacuation (DVE tensor_scalar: acc*scale +
  bias*scale with per-partition vectors), eliminating the baseline's
  rank-1 bias matmuls. All output stores are held in SBUF and issued at
  the end of the body so the weight-read stream is never interleaved with
  HBM writes.
"""

import numpy as np

import concourse.bass as bass
import concourse.mybir as mybir
from concourse import tile
from concourse.bass_utils import run_bass_kernel_spmd

B, IN, OUT = 512, 4096, 11008
N_CORES = 8
O_SH = OUT // N_CORES            # 1376 out-features per core
K_TILES = IN // 128              # 32 contraction chunks
O_TILE_WIDTHS = [128] * (O_SH // 128) + ([O_SH % 128] if O_SH % 128 else [])
N_OT = len(O_TILE_WIDTHS)        # 11 (10x128 + 96)
O_GROUPS = [list(range(0, 4)), list(range(4, 8)), list(range(8, N_OT))]
CHUNK = 4                        # contraction chunks per weight DMA
FP32 = mybir.dt.float32
BF16 = mybir.dt.bfloat16
UINT8 = mybir.dt.uint8
INT8 = mybir.dt.int8

_COMPILED = {}


def _group_geometry():
    o_offs = np.cumsum([0] + O_TILE_WIDTHS).tolist()
    geo = []
    blk_off = 0
    for group in O_GROUPS:
        g0 = o_offs[group[0]]
        gw = o_offs[group[-1] + 1] - g0
        geo.append((group, g0, gw, blk_off))
        blk_off += IN * gw
    return o_offs, geo


def _split_multi_waits(nc: bass.Bass) -> int:
    """The walrus build in this container accepts at most ONE sync wait per
    instruction; Tile freely emits several. Split extras into single-wait
    NoOps on the same engine, inserted just before the instruction
    (semantically identical: all waits must pass before it executes)."""
    n_split = 0
    for blk in nc.main_func.blocks:
        new_insts = []
        for inst in blk.instructions:
            si = inst.sync_info
            if si is not None and len(si.on_wait) > 1:
                waits = list(si.on_wait)
                for w in waits[:-1]:
                    nop = mybir.InstNoOp(
                        name=nc.get_next_instruction_name(), ins=[], outs=[])
                    nop.engine = inst.engine
                    nop.sync_info = mybir.SyncInfo(on_wait=[w], on_update=[])
                    nc.register_instruction(nop)
                    new_insts.append(nop)
                    n_split += 1
                inst.sync_info = mybir.SyncInfo(
                    on_wait=[waits[-1]], on_update=list(si.on_update))
            new_insts.append(inst)
        blk.instructions = new_insts
    return n_split


def _build(c0: float, c1: float, repeat: int = 1, variant: str = "wend",
           stage_bufs: int = 3, unroll: bool = False) -> bass.Bass:
    nc = bass.Bass()
    storedB = nc.dram_tensor("storedB", [IN * O_SH], UINT8, kind="ExternalInput")
    signB = nc.dram_tensor("signB", [IN * O_SH], INT8, kind="ExternalInput")
    xT = nc.dram_tensor("xT", [IN, B], FP32, kind="ExternalInput")
    scale_m = nc.dram_tensor("scale_m", [128, N_OT], FP32, kind="ExternalInput")
    bsc_m = nc.dram_tensor("bsc_m", [128, N_OT], FP32, kind="ExternalInput")
    out = nc.dram_tensor("out", [O_SH, B], FP32, kind="ExternalOutput")

    with tile.TileContext(nc) as tc:
        with (
            tc.tile_pool(name="consts", bufs=1) as consts,
            tc.tile_pool(name="stage", bufs=stage_bufs) as stage,
            tc.tile_pool(name="resp", bufs=3) as resp,
            tc.tile_pool(name="xstage", bufs=2) as xstage,
            tc.tile_pool(name="psum", bufs=1, space="PSUM") as psum,
        ):
            c0_t = consts.tile([128, 1], FP32)
            nc.vector.memset(c0_t[:], c0)
            scale_t = consts.tile([128, N_OT], FP32)
            nc.sync.dma_start(scale_t[:], scale_m[:])
            bsc_t = consts.tile([128, N_OT], FP32)
            nc.sync.dma_start(bsc_t[:], bsc_m[:])

            x_tiles = {}

            def get_x(i):
                # Lazy: emitted at first use so the weight-stream DMAs are
                # not queued behind the full 8.4 MB x preload at kernel
                # start. For repeat>1 all tiles are pre-emitted outside the
                # loop (below), so the loop body slope measures steady state.
                if i not in x_tiles:
                    xf = xstage.tile([128, B], FP32, tag="xf", name=f"xf_{i}")
                    nc.sync.dma_start(xf[:], xT[i * 128:(i + 1) * 128, :])
                    xt = consts.tile([128, B], BF16, tag=f"x{i}", name=f"x_{i}")
                    nc.vector.tensor_copy(xt[:], xf[:])
                    x_tiles[i] = xt
                return x_tiles[i]

            if repeat != 1:
                for i in range(K_TILES):
                    get_x(i)

            o_offs, geo = _group_geometry()

            def body():
                emit_groups(nc, o_offs, geo, storedB, signB, out,
                            get_x, scale_t, bsc_t, c0_t, c1,
                            stage, resp, psum, variant)

            if repeat == 1:
                body()
            elif unroll is True:
                for _ in range(repeat):
                    body()
            elif unroll:
                # unroll U bodies per For_i iteration: the all-engine barrier
                # Tile emits at each loop boundary is paid once per U passes,
                # and consecutive bodies pipeline through the tile-pool WAR
                # dependencies instead of draining every pass.
                assert repeat % unroll == 0
                with tc.For_i(0, repeat // unroll, 1):
                    for _ in range(unroll):
                        body()
            else:
                with tc.For_i(0, repeat, 1):
                    body()

    _split_multi_waits(nc)
    nc.finalize()
    return nc


def emit_groups(nc, o_offs, geo, storedB, signB, out, get_x,
                scale_t, bsc_t, c0_t, c1, stage, resp, psum, variant="wend"):
    late_stores = []

    def make_tail(group, accs):
        # group output path: per-channel scale AND bias applied during the
        # PSUM -> SBUF evacuation (DVE tensor_scalar: acc*scale + bias*scale
        # with per-partition vectors), then store. Emitted AFTER the next
        # group's pipeline has started so in-order engine queues never stall.
        # variant "wend": stores are held until the end of the body so the
        # weight-read stream is never interleaved with HBM writes.
        def tail():
            for t in group:
                tw = O_TILE_WIDTHS[t]
                oo = o_offs[t]
                if variant == "nout":
                    continue
                tag = f"res{t}" if variant == "wend" else "res"
                res = resp.tile([128, B], FP32, tag=tag, name=f"res_{t}")
                nc.vector.tensor_scalar(res[:tw, :], accs[t][:],
                                        scale_t[:tw, t:t + 1],
                                        bsc_t[:tw, t:t + 1],
                                        mybir.AluOpType.mult,
                                        mybir.AluOpType.add)
                if variant == "wend":
                    late_stores.append((oo, tw, res))
                else:
                    nc.sync.dma_start(out[oo:oo + tw, :], res[:tw, :])
        return tail

    pending_tail = None
    for group, g0, gw, blk in geo:
        if variant != "nope":
            accs = {t: psum.tile([O_TILE_WIDTHS[t], B], FP32,
                                 name=f"acc_{t}", tag=f"acc{t % 8}")
                    for t in group}
        for ib in range(K_TILES // CHUNK):
            # one fully-linear DMA covering CHUNK contraction chunks
            span = CHUNK * 128 * gw
            src_st = storedB[blk + ib * span: blk + (ib + 1) * span]
            src_sg = signB[blk + ib * span: blk + (ib + 1) * span]
            st = stage.tile([128, CHUNK, gw], UINT8, tag="st")
            nc.sync.dma_start(st[:], src_st.rearrange("(p a b) -> p a b",
                                                      p=128, b=gw))
            sg = stage.tile([128, CHUNK, gw], INT8, tag="sg")
            nc.sync.dma_start(sg[:], src_sg.rearrange("(p a b) -> p a b",
                                                      p=128, b=gw))
            if variant == "dma":
                continue
            wmag = stage.tile([128, CHUNK, gw], BF16, tag="wmag")
            nc.scalar.activation(wmag[:], st[:], mybir.ActivationFunctionType.Exp,
                                 bias=c0_t[:], scale=c1)
            w = stage.tile([128, CHUNK, gw], BF16, tag="w")
            nc.vector.tensor_mul(w[:], wmag[:], sg[:])
            if variant == "nope":
                continue
            for j in range(CHUNK):
                i = ib * CHUNK + j
                for t in group:
                    tw = O_TILE_WIDTHS[t]
                    toff = o_offs[t] - g0
                    nc.tensor.matmul(
                        accs[t][:],
                        w[:, j, toff:toff + tw],
                        get_x(i)[:],
                        start=(i == 0), stop=(i == K_TILES - 1),
                    )
            if ib == 1 and pending_tail is not None:
                pending_tail()
                pending_tail = None
        if variant in ("nope", "dma"):
            continue
        pending_tail = make_tail(group, accs)
    if pending_tail is not None:
        pending_tail()
    for oo, tw, res in late_stores:
        nc.sync.dma_start(out[oo:oo + tw, :], res[:tw, :])


def _blocked(mT: np.ndarray) -> np.ndarray:
    """[IN, O_SH] -> flat layout where every DMA tile [128, CHUNK*gw] is
    per-partition contiguous: for each group, for each super-chunk ib,
    a [128][CHUNK*gw] block with partition p holding rows
    (ib*CHUNK+j)*128 + p for j in 0..CHUNK."""
    _, geo = _group_geometry()
    parts = []
    for _, g0, gw, _ in geo:
        gcols = mT[:, g0:g0 + gw]              # [IN, gw]
        blk = gcols.reshape(K_TILES // CHUNK, CHUNK, 128, gw)
        parts.append(np.ascontiguousarray(blk.transpose(0, 2, 1, 3)).ravel())
    return np.concatenate(parts)


def kernel(x, stored, sign, log_min, log_max, scale, bias):
    log_min = float(np.asarray(log_min))
    log_max = float(np.asarray(log_max))
    # exp(log_min + (255 - s)/254 * d) == exp(c0 + c1*s)
    d = log_max - log_min
    c1 = -d / 254.0
    c0 = log_min + 255.0 * d / 254.0

    key = (c0, c1)
    if key not in _COMPILED:
        _COMPILED[key] = _build(c0, c1)
    nc = _COMPILED[key]

    xT = np.ascontiguousarray(np.asarray(x, dtype=np.float32).T)
    stored_u8 = np.asarray(stored, dtype=np.uint8)       # values in [1,255]
    sign_i8 = np.asarray(sign, dtype=np.int8)            # values in {-1,+1}
    scale = np.asarray(scale, dtype=np.float32)
    bias = np.asarray(bias, dtype=np.float32)

    in_maps = []
    for c in range(N_CORES):
        o0, o1 = c * O_SH, (c + 1) * O_SH
        scale_pad = np.ones(N_OT * 128, dtype=np.float32)
        scale_pad[:O_SH] = scale[o0:o1]
        bsc_pad = np.zeros(N_OT * 128, dtype=np.float32)
        bsc_pad[:O_SH] = bias[o0:o1] * scale[o0:o1]
        in_maps.append({
            "storedB": _blocked(stored_u8[o0:o1].T),
            "signB": _blocked(sign_i8[o0:o1].T),
            "xT": xT,
            "scale_m": np.ascontiguousarray(scale_pad.reshape(N_OT, 128).T),
            "bsc_m": np.ascontiguousarray(bsc_pad.reshape(N_OT, 128).T),
        })

    global _last_in_maps
    _last_in_maps = in_maps
    res = run_bass_kernel_spmd(nc, in_maps, list(range(N_CORES)))
    yT = np.concatenate([res.results[c]["out"] for c in range(N_CORES)], axis=0)
    return np.ascontiguousarray(yT.T)
